# revision 18
# baseline (speedup 1.0000x reference)
"""CostGlobalEncoder TRN2 kernel: conv3x3(324->128) + global HW x HW attention
+ proj + FFN, data-parallel over batch N=8 across 8 NeuronCores.

Self-contained: hardcodes shapes N=8, D=128, H=48, W=64 (HW=3072).
"""
import sys
sys.path.insert(0, '/opt/trn_rl_repo')

import numpy as np
import ml_dtypes

import concourse.bass as bass
import concourse.tile as tile
from concourse import mybir
from concourse.bass_utils import run_bass_kernel_spmd

N, D, H, W = 8, 128, 48, 64
HW = H * W                    # 3072
CIN = 324                     # corr channels
KC = 108                      # conv contraction chunk (324 = 3*108)
NT = 6                        # i-tiles of 512 positions
NP = NT // 2                  # i-tile pairs
TI = 512                      # positions per i-tile
RT = TI // W                  # 8 rows per i-tile
NJ = HW // 128                # 24 j-tiles
SCALE = float(D) ** -0.5

F32 = mybir.dt.float32
BF16 = mybir.dt.bfloat16
AF = mybir.ActivationFunctionType


def _split_multi_waits(nc, max_waits=1):
    """walrus setupSyncWait rejects instructions with several sem-waits;
    hoist extras onto preceding same-engine NOPs (engines run in order)."""
    for fn in nc.m.functions:
        for blk in fn.blocks:
            insts = blk.instructions
            i = 0
            while i < len(insts):
                inst = insts[i]
                si = inst.sync_info
                if si is not None and si.on_wait and len(si.on_wait) > max_waits:
                    waits = list(si.on_wait)
                    extra, keep = waits[:-max_waits], waits[-max_waits:]
                    nops = []
                    while extra:
                        chunk, extra = extra[:max_waits], extra[max_waits:]
                        nop = mybir.InstNoOp(
                            name=f"waitsplit-{nc.next_id()}", ins=[], outs=[])
                        nop.engine = inst.engine
                        nop.sync_info = mybir.SyncInfo(on_wait=chunk, on_update=[])
                        nops.append(nop)
                    inst.sync_info = mybir.SyncInfo(
                        on_wait=keep, on_update=list(si.on_update))
                    blk.instructions = insts = insts[:i] + nops + insts[i:]
                    i += len(nops)
                i += 1


def build_nc():
    nc = bass.Bass()
    corr = nc.declare_dram_parameter("corr", [CIN, HW], BF16, isOutput=False)
    k_in = nc.declare_dram_parameter("k", [D, HW], BF16, isOutput=False)
    vT = nc.declare_dram_parameter("vT", [128, NJ, D], BF16, isOutput=False)
    wskT = nc.declare_dram_parameter("wskT", [KC, 27, D], BF16, isOutput=False)
    b_sk = nc.declare_dram_parameter("b_sk", [D, 1], F32, isOutput=False)
    wprojT = nc.declare_dram_parameter("wprojT", [2, D, D], BF16, isOutput=False)
    b_proj = nc.declare_dram_parameter("b_proj", [1, D], BF16, isOutput=False)
    wf1T = nc.declare_dram_parameter("wf1T", [D, D], BF16, isOutput=False)
    b_f1 = nc.declare_dram_parameter("b_f1", [D, 1], F32, isOutput=False)
    wf2T = nc.declare_dram_parameter("wf2T", [D, D], BF16, isOutput=False)
    b_f2 = nc.declare_dram_parameter("b_f2", [1, D], BF16, isOutput=False)
    out = nc.declare_dram_parameter("out", [D, HW], F32, isOutput=True)

    with tile.TileContext(nc) as tc:
        with (
            tc.tile_pool(name="const", bufs=1) as cpool,
            tc.tile_pool(name="stage", bufs=2) as spool,
            tc.tile_pool(name="work", bufs=2) as wpool,
            tc.tile_pool(name="qpool", bufs=4) as qpool,
            tc.tile_pool(name="xpool", bufs=7) as xpool,
            tc.tile_pool(name="epool", bufs=50) as epool,
            tc.tile_pool(name="ps_s", bufs=2, space="PSUM") as ps_s,
            tc.tile_pool(name="ps_av", bufs=2, space="PSUM") as ps_av,
            tc.tile_pool(name="ps_conv", bufs=1, space="PSUM") as ps_conv,
            tc.tile_pool(name="ps_small", bufs=2, space="PSUM") as ps_small,
        ):
            # ---- load inputs: wskT + corr chunk 0 gate the first matmul ----
            wskT_sb = cpool.tile([KC, 27, D], BF16)
            for c in range(3):
                nc.sync.dma_start(wskT_sb[:, c * 9:(c + 1) * 9, :],
                                  wskT[:, c * 9:(c + 1) * 9, :])
            b_sk_sb = cpool.tile([D, 1], F32)
            nc.sync.dma_start(b_sk_sb[:], b_sk[:])
            corr_pad = []
            for c in range(3):
                cp = cpool.tile([KC, H + 2, W + 2], BF16, name=f"corr_pad{c}")
                nc.vector.memset(cp[:, 0, :], 0.0)
                nc.vector.memset(cp[:, H + 1, :], 0.0)
                nc.vector.memset(cp[:, 1:H + 1, 0:1], 0.0)
                nc.vector.memset(cp[:, 1:H + 1, W + 1:W + 2], 0.0)
                stg = spool.tile([KC, HW], BF16, name="stg")
                nc.sync.dma_start(stg[:], corr[c * KC:(c + 1) * KC, :])
                nc.vector.tensor_copy(
                    cp[:, 1:H + 1, 1:W + 1],
                    stg.rearrange("p (h w) -> p h w", h=H))
                corr_pad.append(cp)
            k_sb = cpool.tile([D, HW], BF16)
            nc.gpsimd.dma_start(k_sb[:], k_in[:])
            # vT_sb[p, t, d] = v[d, t*128+p]
            vT_sb = cpool.tile([128, NJ, D], BF16)
            nc.gpsimd.dma_start(vT_sb[:], vT[:])
            wprojT_sb = cpool.tile([D, 2, D], BF16)
            nc.gpsimd.dma_start(wprojT_sb[:], wprojT.rearrange("c p d -> p c d"))
            wf1T_sb = cpool.tile([D, D], BF16)
            nc.gpsimd.dma_start(wf1T_sb[:], wf1T[:])
            wf2T_sb = cpool.tile([D, D], BF16)
            nc.gpsimd.dma_start(wf2T_sb[:], wf2T[:])
            b_proj_sb = cpool.tile([1, D], BF16)
            nc.gpsimd.dma_start(b_proj_sb[:], b_proj[:])
            b_f1_sb = cpool.tile([D, 1], F32)
            nc.gpsimd.dma_start(b_f1_sb[:], b_f1[:])
            b_f2_sb = cpool.tile([1, D], BF16)
            nc.gpsimd.dma_start(b_f2_sb[:], b_f2[:])
            ones_bf = cpool.tile([128, 1], BF16)
            nc.gpsimd.memset(ones_bf[:], 1.0)
            ones_row = cpool.tile([1, TI], BF16)
            nc.gpsimd.memset(ones_row[:], 1.0)
            ones_bf4 = cpool.tile([4, 128], BF16)
            nc.gpsimd.memset(ones_bf4[:], 1.0)

            def conv_mm(p):
                """3x3 conv matmuls for i-tiles (2p, 2p+1); weight-paired."""
                ps_c = ps_conv.tile([D, 2, TI], F32, name="ps_c")
                for c in range(3):
                    for t in range(9):
                        dy, dx = t // 3, t % 3
                        for ii in range(2):
                            y0 = (2 * p + ii) * RT
                            nc.tensor.matmul(
                                ps_c[:, ii, :],
                                wskT_sb[:, c * 9 + t, :],
                                corr_pad[c][:, y0 + dy:y0 + dy + RT,
                                            dx:dx + W],
                                start=(c == 0 and t == 0),
                                stop=(c == 2 and t == 8))
                return ps_c

            def conv_evac(ps_c):
                rq = []
                for ii in range(2):
                    resid = qpool.tile([D, TI], F32, name="resid")
                    nc.vector.tensor_scalar_add(resid[:], ps_c[:, ii, :],
                                                b_sk_sb[:])
                    q = qpool.tile([D, TI], BF16, name="q")
                    nc.vector.tensor_copy(q[:], resid[:])
                    rq.append((resid, q))
                return rq

            def s_pair(j, qs):
                ps_sj = []
                for ii in range(2):
                    t = ps_s.tile([128, TI], F32, name="ps_sj", tag="s")
                    nc.tensor.matmul(t[:], k_sb[:, j * 128:(j + 1) * 128],
                                     qs[ii][1][:], start=True, stop=True)
                    ps_sj.append(t)
                return ps_sj

            def normalize_a(ps_m):
                """evacuate the 4 packed denominator rows to SBUF."""
                m4 = wpool.tile([128, TI], BF16, name="m4")
                nc.vector.tensor_copy(m4[:], ps_m[:])
                return m4

            def normalize_b(m4, ps_a):
                """merge+broadcast denominators via gathered [4,TI] matmul."""
                m4g = wpool.tile([4, TI], BF16, name="m4g")
                for g in range(4):
                    nc.sync.dma_start(m4g[g:g + 1, :],
                                      m4[32 * g:32 * g + 1, :])
                ps_b = ps_small.tile([128, TI], F32, name="ps_b", tag="sm")
                nc.tensor.matmul(ps_b[:], ones_bf4[:], m4g[:],
                                 start=True, stop=True)
                rb = wpool.tile([128, TI], F32, name="rb")
                nc.vector.reciprocal(rb[:], ps_b[:])
                attn = wpool.tile([D, TI], BF16, name="attn")
                nc.vector.tensor_mul(attn[:], ps_a[:], rb[:])
                return attn

            def proj(attn, resid, q):
                """1x1 proj on concat([attn, resid]) + bias + resid."""
                ps_p = ps_av.tile([D, TI], F32, name="ps_p", tag="av")
                nc.tensor.matmul(ps_p[:], wprojT_sb[:, 0, :], attn[:],
                                 start=True, stop=False)
                nc.tensor.matmul(ps_p[:], wprojT_sb[:, 1, :], q[:],
                                 start=False, stop=False)
                nc.tensor.matmul(ps_p[:], b_proj_sb[:], ones_row[:],
                                 start=False, stop=True)
                x = xpool.tile([D, TI], F32, name="x")
                nc.vector.tensor_add(x[:], ps_p[:], resid[:])
                x_bf = xpool.tile([D, TI], BF16, name="x_bf")
                nc.vector.tensor_copy(x_bf[:], x[:])
                return x, x_bf

            def ffn(xv, i):
                x, x_bf = xv
                ps_f1 = ps_s.tile([D, TI], F32, name="ps_f1", tag="s")
                nc.tensor.matmul(ps_f1[:], wf1T_sb[:], x_bf[:],
                                 start=True, stop=True)
                h1 = wpool.tile([D, TI], BF16, name="h1")
                nc.scalar.activation(h1[:], ps_f1[:], AF.Gelu, bias=b_f1_sb[:])
                ps_f2 = ps_s.tile([D, TI], F32, name="ps_f2", tag="s")
                nc.tensor.matmul(ps_f2[:], wf2T_sb[:], h1[:],
                                 start=True, stop=False)
                nc.tensor.matmul(ps_f2[:], b_f2_sb[:], ones_row[:],
                                 start=False, stop=True)
                o = wpool.tile([D, TI], F32, name="o")
                nc.vector.tensor_add(o[:], ps_f2[:], x[:])
                nc.gpsimd.dma_start(out[:, i * TI:(i + 1) * TI], o[:])

            xs = [None] * NT
            rq_pair = conv_evac(conv_mm(0))
            for p in range(NP):
                i0, i1 = 2 * p, 2 * p + 1
                # ---- attention j-loop, software-pipelined by one j ----
                ps_a0 = ps_av.tile([D, TI], F32, name="ps_a0", tag="av")
                ps_a1 = ps_av.tile([D, TI], F32, name="ps_a1", tag="av")
                ps_sj = s_pair(0, rq_pair)
                e_tiles = [[], []]
                for j in range(NJ):
                    ps_nxt = s_pair(j + 1, rq_pair) if j + 1 < NJ else None
                    for ii, ps_aa in ((0, ps_a0), (1, ps_a1)):
                        e = epool.tile([128, TI], BF16, name="e")
                        nc.scalar.activation(e[:], ps_sj[ii][:], AF.Exp,
                                             scale=SCALE)
                        nc.tensor.matmul(ps_aa[:], vT_sb[:, j, :], e[:],
                                         start=(j == 0), stop=(j == NJ - 1))
                        e_tiles[ii].append(e)
                    ps_sj = ps_nxt

                # ---- softmax denominators: col-packed ones-matmuls ----
                m4gs = []
                for ii in range(2):
                    ps_m = ps_small.tile([128, TI], F32, name="ps_m", tag="sm")
                    for g4 in range(NJ // 4):
                        for g in range(4):
                            nc.tensor.matmul(
                                ps_m[32 * g:32 * g + 1, :], ones_bf[:, 0:1],
                                e_tiles[ii][g4 * 4 + g][:],
                                start=(g4 == 0), stop=(g4 == NJ // 4 - 1),
                                tile_position=(0, 32 * g))
                    m4gs.append(normalize_a(ps_m))

                rq_prev = rq_pair
                ps_c_next = conv_mm(p + 1) if p + 1 < NP else None

                if p == NP - 1:
                    # cover the exposed last-pair tail with FFN PE work
                    for i in range(2 * (NP - 1)):
                        ffn(xs[i], i)

                for ii, ps_aa in ((0, ps_a0), (1, ps_a1)):
                    attn = normalize_b(m4gs[ii], ps_aa)
                    resid, q = rq_prev[ii]
                    xs[2 * p + ii] = proj(attn, resid, q)

                if ps_c_next is not None:
                    rq_pair = conv_evac(ps_c_next)

            # ---- FFN for the last pair ----
            for i in range(2 * (NP - 1), NT):
                ffn(xs[i], i)

    _split_multi_waits(nc)
    return nc


_NC = None


def _get_nc():
    global _NC
    if _NC is None:
        _NC = build_nc()
    return _NC


def _prep_core(corr, k, v, w_sk, b_sk, w_proj, b_proj, w_ffn1, b_ffn1,
               w_ffn2, b_ffn2):
    bf = ml_dtypes.bfloat16
    wskT = np.empty((KC, 27, D), dtype=bf)
    for c in range(3):
        for t in range(9):
            dy, dx = t // 3, t % 3
            wskT[:, c * 9 + t, :] = \
                w_sk[:, c * KC:(c + 1) * KC, dy, dx].T.astype(bf)
    vT = v.reshape(D, HW).T.reshape(NJ, 128, D).transpose(1, 0, 2)
    return {
        "corr": corr.reshape(CIN, HW).astype(bf),
        "k": k.reshape(D, HW).astype(bf),
        "vT": np.ascontiguousarray(vT).astype(bf),
        "wskT": wskT,
        "b_sk": b_sk.reshape(D, 1).astype(np.float32),
        "wprojT": np.ascontiguousarray(
            w_proj.reshape(D, 2 * D).T.reshape(2, D, D)).astype(bf),
        "b_proj": b_proj.reshape(1, D).astype(bf),
        "wf1T": np.ascontiguousarray(w_ffn1.reshape(D, D).T).astype(bf),
        "b_f1": b_ffn1.reshape(D, 1).astype(np.float32),
        "wf2T": np.ascontiguousarray(w_ffn2.reshape(D, D).T).astype(bf),
        "b_f2": b_ffn2.reshape(1, D).astype(bf),
    }


def make_in_maps(corr, k, v, w_sk, b_sk, w_proj, b_proj, w_ffn1, b_ffn1,
                 w_ffn2, b_ffn2):
    corr = np.asarray(corr, dtype=np.float32)
    k = np.asarray(k, dtype=np.float32)
    v = np.asarray(v, dtype=np.float32)
    return [
        _prep_core(corr[i], k[i], v[i], np.asarray(w_sk, np.float32),
                   np.asarray(b_sk, np.float32),
                   np.asarray(w_proj, np.float32),
                   np.asarray(b_proj, np.float32),
                   np.asarray(w_ffn1, np.float32),
                   np.asarray(b_ffn1, np.float32),
                   np.asarray(w_ffn2, np.float32),
                   np.asarray(b_ffn2, np.float32))
        for i in range(N)
    ]


def kernel(corr, k, v, w_sk, b_sk, w_proj, b_proj, w_ffn1, b_ffn1,
           w_ffn2, b_ffn2):
    nc = _get_nc()
    in_maps = make_in_maps(corr, k, v, w_sk, b_sk, w_proj, b_proj,
                           w_ffn1, b_ffn1, w_ffn2, b_ffn2)
    res = run_bass_kernel_spmd(nc, in_maps, list(range(N)))
    out = np.stack([res.results[i]["out"].reshape(D, H, W) for i in range(N)])
    return out.astype(np.float32)


# revision 25
# speedup vs baseline: 1.1413x; 1.1413x over previous
"""CostGlobalEncoder TRN2 kernel: conv3x3(324->128) + global HW x HW attention
+ proj + FFN, data-parallel over batch N=8 across 8 NeuronCores.

Self-contained: hardcodes shapes N=8, D=128, H=48, W=64 (HW=3072).
"""
import sys
sys.path.insert(0, '/opt/trn_rl_repo')

import numpy as np
import ml_dtypes

import concourse.bass as bass
import concourse.tile as tile
from concourse import mybir
from concourse.bass_utils import run_bass_kernel_spmd

N, D, H, W = 8, 128, 48, 64
HW = H * W                    # 3072
CIN = 324                     # corr channels
KC = 108                      # conv contraction chunk (324 = 3*108)
NT = 6                        # i-tiles of 512 positions
NP = NT // 2                  # i-tile pairs
TI = 512                      # positions per i-tile
RT = TI // W                  # 8 rows per i-tile
NJ = HW // 128                # 24 j-tiles
SCALE = float(D) ** -0.5

F32 = mybir.dt.float32
BF16 = mybir.dt.bfloat16
AF = mybir.ActivationFunctionType


def _split_multi_waits(nc, max_waits=1):
    """walrus setupSyncWait rejects instructions with several sem-waits;
    hoist extras onto preceding same-engine NOPs (engines run in order)."""
    for fn in nc.m.functions:
        for blk in fn.blocks:
            insts = blk.instructions
            i = 0
            while i < len(insts):
                inst = insts[i]
                si = inst.sync_info
                if si is not None and si.on_wait and len(si.on_wait) > max_waits:
                    waits = list(si.on_wait)
                    extra, keep = waits[:-max_waits], waits[-max_waits:]
                    nops = []
                    while extra:
                        chunk, extra = extra[:max_waits], extra[max_waits:]
                        nop = mybir.InstNoOp(
                            name=f"waitsplit-{nc.next_id()}", ins=[], outs=[])
                        nop.engine = inst.engine
                        nop.sync_info = mybir.SyncInfo(on_wait=chunk, on_update=[])
                        nops.append(nop)
                    inst.sync_info = mybir.SyncInfo(
                        on_wait=keep, on_update=list(si.on_update))
                    blk.instructions = insts = insts[:i] + nops + insts[i:]
                    i += len(nops)
                i += 1


def build_nc(with_bias=True):
    nc = bass.Bass()
    corr = nc.declare_dram_parameter("corr", [CIN, HW], BF16, isOutput=False)
    k_in = nc.declare_dram_parameter("k", [D, HW], BF16, isOutput=False)
    vT = nc.declare_dram_parameter("vT", [128, NJ, D], BF16, isOutput=False)
    wskT = nc.declare_dram_parameter("wskT", [KC, 27, D], BF16, isOutput=False)
    b_sk = nc.declare_dram_parameter("b_sk", [D, 1], F32, isOutput=False)
    wprojT = nc.declare_dram_parameter("wprojT", [2, D, D], BF16, isOutput=False)
    b_proj = nc.declare_dram_parameter("b_proj", [1, D], BF16, isOutput=False)
    wf1T = nc.declare_dram_parameter("wf1T", [D, D], BF16, isOutput=False)
    b_f1 = nc.declare_dram_parameter("b_f1", [D, 1], F32, isOutput=False)
    wf2T = nc.declare_dram_parameter("wf2T", [D, D], BF16, isOutput=False)
    b_f2 = nc.declare_dram_parameter("b_f2", [1, D], BF16, isOutput=False)
    out = nc.declare_dram_parameter("out", [D, HW], F32, isOutput=True)

    with tile.TileContext(nc) as tc:
        with (
            tc.tile_pool(name="const", bufs=1) as cpool,
            tc.tile_pool(name="stage", bufs=2) as spool,
            tc.tile_pool(name="work", bufs=2) as wpool,
            tc.tile_pool(name="qpool", bufs=4) as qpool,
            tc.tile_pool(name="xpool", bufs=7) as xpool,
            tc.tile_pool(name="epool", bufs=50) as epool,
            tc.tile_pool(name="ps_s", bufs=3, space="PSUM") as ps_s,
            tc.tile_pool(name="ps_av", bufs=2, space="PSUM") as ps_av,
            tc.tile_pool(name="ps_conv", bufs=1, space="PSUM") as ps_conv,
            tc.tile_pool(name="ps_small", bufs=1, space="PSUM") as ps_small,
        ):
            # ---- load inputs: wskT + corr chunk 0 gate the first matmul ----
            wskT_sb = cpool.tile([KC, 27, D], BF16)
            for c in range(3):
                nc.sync.dma_start(wskT_sb[:, c * 9:(c + 1) * 9, :],
                                  wskT[:, c * 9:(c + 1) * 9, :])
            b_sk_sb = cpool.tile([D, 1], F32)
            nc.sync.dma_start(b_sk_sb[:], b_sk[:])
            corr_pad = []
            for c in range(3):
                cp = cpool.tile([KC, H + 2, W + 2], BF16, name=f"corr_pad{c}")
                nc.vector.memset(cp[:, 0, :], 0.0)
                nc.vector.memset(cp[:, H + 1, :], 0.0)
                nc.vector.memset(cp[:, 1:H + 1, 0:1], 0.0)
                nc.vector.memset(cp[:, 1:H + 1, W + 1:W + 2], 0.0)
                stg = spool.tile([KC, HW], BF16, name="stg")
                nc.sync.dma_start(stg[:], corr[c * KC:(c + 1) * KC, :])
                nc.vector.tensor_copy(
                    cp[:, 1:H + 1, 1:W + 1],
                    stg.rearrange("p (h w) -> p h w", h=H))
                corr_pad.append(cp)
            k_sb = cpool.tile([D, HW], BF16)
            nc.sync.dma_start(k_sb[:], k_in[:])
            # vT_sb[p, t, d] = v[d, t*128+p]
            vT_sb = cpool.tile([128, NJ, D], BF16)
            nc.sync.dma_start(vT_sb[:], vT[:])
            wprojT_sb = cpool.tile([D, 2, D], BF16)
            nc.gpsimd.dma_start(wprojT_sb[:], wprojT.rearrange("c p d -> p c d"))
            wf1T_sb = cpool.tile([D, D], BF16)
            nc.gpsimd.dma_start(wf1T_sb[:], wf1T[:])
            wf2T_sb = cpool.tile([D, D], BF16)
            nc.gpsimd.dma_start(wf2T_sb[:], wf2T[:])
            b_proj_sb = cpool.tile([1, D], BF16)
            nc.gpsimd.dma_start(b_proj_sb[:], b_proj[:])
            b_f1_sb = cpool.tile([D, 1], F32)
            nc.gpsimd.dma_start(b_f1_sb[:], b_f1[:])
            b_f2_sb = cpool.tile([1, D], BF16)
            nc.gpsimd.dma_start(b_f2_sb[:], b_f2[:])
            ones_bf = cpool.tile([128, 1], BF16)
            nc.gpsimd.memset(ones_bf[:], 1.0)
            ones_row = cpool.tile([1, TI], BF16)
            nc.gpsimd.memset(ones_row[:], 1.0)
            ones_bf4 = cpool.tile([4, 128], BF16)
            nc.gpsimd.memset(ones_bf4[:], 1.0)
            warm = cpool.tile([128, 128], BF16)
            nc.gpsimd.memset(warm[:], 0.0)
            # HAM warm-up: keep PE busy during the input DMA wait so the
            # first conv runs at 2.4 GHz (clock-gate releases after ~3.4us)
            ps_w = ps_small.tile([128, 128], F32, name="ps_w", tag="sm")
            for _ in range(72):
                nc.tensor.matmul(ps_w[:], warm[:], warm[:],
                                 start=True, stop=True)

            def conv_mm(p):
                """3x3 conv matmuls for i-tiles (2p, 2p+1); weight-paired."""
                ps_c = ps_conv.tile([D, 2, TI], F32, name="ps_c")
                for c in range(3):
                    for t in range(9):
                        dy, dx = t // 3, t % 3
                        for ii in range(2):
                            y0 = (2 * p + ii) * RT
                            nc.tensor.matmul(
                                ps_c[:, ii, :],
                                wskT_sb[:, c * 9 + t, :],
                                corr_pad[c][:, y0 + dy:y0 + dy + RT,
                                            dx:dx + W],
                                start=(c == 0 and t == 0),
                                stop=(c == 2 and t == 8))
                return ps_c

            def conv_evac(ps_c):
                rq = []
                for ii in range(2):
                    resid = qpool.tile([D, TI], F32, name="resid")
                    nc.vector.tensor_scalar_add(resid[:], ps_c[:, ii, :],
                                                b_sk_sb[:])
                    q = qpool.tile([D, TI], BF16, name="q")
                    nc.vector.tensor_copy(q[:], resid[:])
                    rq.append((resid, q))
                return rq

            def s_pair(j, qs):
                ps_sj = []
                for ii in range(2):
                    t = ps_s.tile([128, TI], F32, name="ps_sj", tag="s")
                    nc.tensor.matmul(t[:], k_sb[:, j * 128:(j + 1) * 128],
                                     qs[ii][1][:], start=True, stop=True)
                    ps_sj.append(t)
                return ps_sj

            def normalize_a(ps_m, ii):
                """evacuate + gather the 4 packed denominator rows."""
                m4 = wpool.tile([128, TI], BF16, name="m4")
                nc.vector.tensor_copy(m4[:], ps_m[:])
                m4g = wpool.tile([4, TI], BF16, name="m4g")
                for g in range(4):
                    nc.sync.dma_start(m4g[g:g + 1, :],
                                      m4[32 * g:32 * g + 1, :])
                return m4g

            def normalize_b(m4g, ps_a):
                """merge+broadcast denominators via gathered [2,TI] matmul."""
                ps_b = ps_small.tile([128, TI], F32, name="ps_b", tag="sm")
                nc.tensor.matmul(ps_b[:], ones_bf4[:], m4g[:],
                                 start=True, stop=True)
                rb = wpool.tile([128, TI], F32, name="rb")
                nc.vector.reciprocal(rb[:], ps_b[:])
                attn = wpool.tile([D, TI], BF16, name="attn")
                nc.vector.tensor_mul(attn[:], ps_a[:], rb[:])
                return attn

            def proj(attn, resid, q):
                """1x1 proj on concat([attn, resid]) + bias + resid."""
                ps_p = ps_av.tile([D, TI], F32, name="ps_p", tag="av")
                nc.tensor.matmul(ps_p[:], wprojT_sb[:, 0, :], attn[:],
                                 start=True, stop=False)
                nc.tensor.matmul(ps_p[:], wprojT_sb[:, 1, :], q[:],
                                 start=False, stop=not with_bias)
                if with_bias:
                    nc.tensor.matmul(ps_p[:], b_proj_sb[:], ones_row[:],
                                     start=False, stop=True)
                x = xpool.tile([D, TI], F32, name="x")
                nc.vector.tensor_add(x[:], ps_p[:], resid[:])
                x_bf = xpool.tile([D, TI], BF16, name="x_bf")
                nc.vector.tensor_copy(x_bf[:], x[:])
                return x, x_bf

            def ffn(xv, i):
                x, x_bf = xv
                ps_f1 = ps_s.tile([D, TI], F32, name="ps_f1", tag="s")
                nc.tensor.matmul(ps_f1[:], wf1T_sb[:], x_bf[:],
                                 start=True, stop=True)
                h1 = wpool.tile([D, TI], BF16, name="h1")
                nc.scalar.activation(h1[:], ps_f1[:], AF.Gelu, bias=b_f1_sb[:])
                ps_f2 = ps_s.tile([D, TI], F32, name="ps_f2", tag="s")
                nc.tensor.matmul(ps_f2[:], wf2T_sb[:], h1[:],
                                 start=True, stop=not with_bias)
                if with_bias:
                    nc.tensor.matmul(ps_f2[:], b_f2_sb[:], ones_row[:],
                                     start=False, stop=True)
                o = wpool.tile([D, TI], F32, name="o")
                nc.vector.tensor_add(o[:], ps_f2[:], x[:])
                nc.gpsimd.dma_start(out[:, i * TI:(i + 1) * TI], o[:])

            xs = [None] * NT
            rq_pair = conv_evac(conv_mm(0))
            for p in range(NP):
                i0, i1 = 2 * p, 2 * p + 1
                # ---- attention j-loop, software-pipelined by one j ----
                ps_a0 = ps_av.tile([D, TI], F32, name="ps_a0", tag="av")
                ps_a1 = ps_av.tile([D, TI], F32, name="ps_a1", tag="av")
                ps_sj = s_pair(0, rq_pair)
                e_tiles = [[], []]
                for j in range(NJ):
                    ps_nxt = s_pair(j + 1, rq_pair) if j + 1 < NJ else None
                    for ii, ps_aa in ((0, ps_a0), (1, ps_a1)):
                        e = epool.tile([128, TI], BF16, name="e")
                        nc.scalar.activation(e[:], ps_sj[ii][:], AF.Exp,
                                             scale=SCALE)
                        nc.tensor.matmul(ps_aa[:], vT_sb[:, j, :], e[:],
                                         start=(j == 0), stop=(j == NJ - 1))
                        e_tiles[ii].append(e)
                    ps_sj = ps_nxt

                # ---- softmax denominators: col-packed ones-matmuls ----
                m4gs = []
                for ii in range(2):
                    ps_m = ps_small.tile([128, TI], F32, name="ps_m", tag="sm")
                    for g4 in range(NJ // 4):
                        for g in range(4):
                            nc.tensor.matmul(
                                ps_m[32 * g:32 * g + 1, :], ones_bf[:, 0:1],
                                e_tiles[ii][g4 * 4 + g][:],
                                start=(g4 == 0), stop=(g4 == NJ // 4 - 1),
                                tile_position=(0, 32 * g))
                    m4gs.append(normalize_a(ps_m, ii))

                rq_prev = rq_pair
                if p + 1 < NP:
                    rq_pair = conv_evac(conv_mm(p + 1))

                last = p == NP - 1
                attn0 = normalize_b(m4gs[0], ps_a0)
                if last:
                    ffn(xs[0], 0)
                    ffn(xs[1], 1)
                xs[2 * p] = proj(attn0, rq_prev[0][0], rq_prev[0][1])
                attn1 = normalize_b(m4gs[1], ps_a1)
                if last:
                    ffn(xs[2], 2)
                    ffn(xs[3], 3)
                xs[2 * p + 1] = proj(attn1, rq_prev[1][0], rq_prev[1][1])

            # ---- FFN for the last pair ----
            for i in range(2 * (NP - 1), NT):
                ffn(xs[i], i)

    _split_multi_waits(nc)
    return nc


_NC = {}


def _get_nc(with_bias=True):
    if with_bias not in _NC:
        _NC[with_bias] = build_nc(with_bias)
    return _NC[with_bias]


def _prep_core(corr, k, v, w_sk, b_sk, w_proj, b_proj, w_ffn1, b_ffn1,
               w_ffn2, b_ffn2):
    bf = ml_dtypes.bfloat16
    wskT = np.empty((KC, 27, D), dtype=bf)
    for c in range(3):
        for t in range(9):
            dy, dx = t // 3, t % 3
            wskT[:, c * 9 + t, :] = \
                w_sk[:, c * KC:(c + 1) * KC, dy, dx].T.astype(bf)
    vT = v.reshape(D, HW).T.reshape(NJ, 128, D).transpose(1, 0, 2)
    return {
        "corr": corr.reshape(CIN, HW).astype(bf),
        "k": k.reshape(D, HW).astype(bf),
        "vT": np.ascontiguousarray(vT).astype(bf),
        "wskT": wskT,
        "b_sk": b_sk.reshape(D, 1).astype(np.float32),
        "wprojT": np.ascontiguousarray(
            w_proj.reshape(D, 2 * D).T.reshape(2, D, D)).astype(bf),
        "b_proj": b_proj.reshape(1, D).astype(bf),
        "wf1T": np.ascontiguousarray(w_ffn1.reshape(D, D).T).astype(bf),
        "b_f1": b_ffn1.reshape(D, 1).astype(np.float32),
        "wf2T": np.ascontiguousarray(w_ffn2.reshape(D, D).T).astype(bf),
        "b_f2": b_ffn2.reshape(1, D).astype(bf),
    }


def make_in_maps(corr, k, v, w_sk, b_sk, w_proj, b_proj, w_ffn1, b_ffn1,
                 w_ffn2, b_ffn2):
    corr = np.asarray(corr, dtype=np.float32)
    k = np.asarray(k, dtype=np.float32)
    v = np.asarray(v, dtype=np.float32)
    return [
        _prep_core(corr[i], k[i], v[i], np.asarray(w_sk, np.float32),
                   np.asarray(b_sk, np.float32),
                   np.asarray(w_proj, np.float32),
                   np.asarray(b_proj, np.float32),
                   np.asarray(w_ffn1, np.float32),
                   np.asarray(b_ffn1, np.float32),
                   np.asarray(w_ffn2, np.float32),
                   np.asarray(b_ffn2, np.float32))
        for i in range(N)
    ]


def kernel(corr, k, v, w_sk, b_sk, w_proj, b_proj, w_ffn1, b_ffn1,
           w_ffn2, b_ffn2):
    with_bias = bool(np.any(np.asarray(b_proj)) or np.any(np.asarray(b_ffn2)))
    nc = _get_nc(with_bias)
    in_maps = make_in_maps(corr, k, v, w_sk, b_sk, w_proj, b_proj,
                           w_ffn1, b_ffn1, w_ffn2, b_ffn2)
    res = run_bass_kernel_spmd(nc, in_maps, list(range(N)))
    out = np.stack([res.results[i]["out"].reshape(D, H, W) for i in range(N)])
    return out.astype(np.float32)


# revision 26
# speedup vs baseline: 1.1466x; 1.0046x over previous
"""CostGlobalEncoder TRN2 kernel: conv3x3(324->128) + global HW x HW attention
+ proj + FFN, data-parallel over batch N=8 across 8 NeuronCores.

Self-contained: hardcodes shapes N=8, D=128, H=48, W=64 (HW=3072).
"""
import sys
sys.path.insert(0, '/opt/trn_rl_repo')

import numpy as np
import ml_dtypes

import concourse.bass as bass
import concourse.tile as tile
from concourse import mybir
from concourse.bass_utils import run_bass_kernel_spmd

N, D, H, W = 8, 128, 48, 64
HW = H * W                    # 3072
CIN = 324                     # corr channels
KC = 108                      # conv contraction chunk (324 = 3*108)
NT = 6                        # i-tiles of 512 positions
NP = NT // 2                  # i-tile pairs
TI = 512                      # positions per i-tile
RT = TI // W                  # 8 rows per i-tile
NJ = HW // 128                # 24 j-tiles
SCALE = float(D) ** -0.5

F32 = mybir.dt.float32
BF16 = mybir.dt.bfloat16
AF = mybir.ActivationFunctionType


def _split_multi_waits(nc, max_waits=1):
    """walrus setupSyncWait rejects instructions with several sem-waits;
    hoist extras onto preceding same-engine NOPs (engines run in order)."""
    for fn in nc.m.functions:
        for blk in fn.blocks:
            insts = blk.instructions
            i = 0
            while i < len(insts):
                inst = insts[i]
                si = inst.sync_info
                if si is not None and si.on_wait and len(si.on_wait) > max_waits:
                    waits = list(si.on_wait)
                    extra, keep = waits[:-max_waits], waits[-max_waits:]
                    nops = []
                    while extra:
                        chunk, extra = extra[:max_waits], extra[max_waits:]
                        nop = mybir.InstNoOp(
                            name=f"waitsplit-{nc.next_id()}", ins=[], outs=[])
                        nop.engine = inst.engine
                        nop.sync_info = mybir.SyncInfo(on_wait=chunk, on_update=[])
                        nops.append(nop)
                    inst.sync_info = mybir.SyncInfo(
                        on_wait=keep, on_update=list(si.on_update))
                    blk.instructions = insts = insts[:i] + nops + insts[i:]
                    i += len(nops)
                i += 1


def build_nc(with_bias=True):
    nc = bass.Bass()
    corr = nc.declare_dram_parameter("corr", [CIN, HW], BF16, isOutput=False)
    k_in = nc.declare_dram_parameter("k", [D, HW], BF16, isOutput=False)
    vT = nc.declare_dram_parameter("vT", [128, NJ, D], BF16, isOutput=False)
    wskT = nc.declare_dram_parameter("wskT", [KC, 27, D], BF16, isOutput=False)
    b_sk = nc.declare_dram_parameter("b_sk", [D, 1], F32, isOutput=False)
    wprojT = nc.declare_dram_parameter("wprojT", [2, D, D], BF16, isOutput=False)
    b_proj = nc.declare_dram_parameter("b_proj", [1, D], BF16, isOutput=False)
    wf1T = nc.declare_dram_parameter("wf1T", [D, D], BF16, isOutput=False)
    b_f1 = nc.declare_dram_parameter("b_f1", [D, 1], F32, isOutput=False)
    wf2T = nc.declare_dram_parameter("wf2T", [D, D], BF16, isOutput=False)
    b_f2 = nc.declare_dram_parameter("b_f2", [1, D], BF16, isOutput=False)
    out = nc.declare_dram_parameter("out", [D, HW], F32, isOutput=True)

    with tile.TileContext(nc) as tc:
        with (
            tc.tile_pool(name="const", bufs=1) as cpool,
            tc.tile_pool(name="stage", bufs=2) as spool,
            tc.tile_pool(name="work", bufs=2) as wpool,
            tc.tile_pool(name="qpool", bufs=6) as qpool,
            tc.tile_pool(name="xpool", bufs=7) as xpool,
            tc.tile_pool(name="epool", bufs=50) as epool,
            tc.tile_pool(name="ps_s", bufs=3, space="PSUM") as ps_s,
            tc.tile_pool(name="ps_av", bufs=2, space="PSUM") as ps_av,
            tc.tile_pool(name="ps_conv", bufs=1, space="PSUM") as ps_conv,
            tc.tile_pool(name="ps_small", bufs=1, space="PSUM") as ps_small,
        ):
            # ---- load inputs: wskT + corr chunk 0 gate the first matmul ----
            wskT_sb = cpool.tile([KC, 27, D], BF16)
            for c in range(3):
                nc.sync.dma_start(wskT_sb[:, c * 9:(c + 1) * 9, :],
                                  wskT[:, c * 9:(c + 1) * 9, :])
            b_sk_sb = cpool.tile([D, 1], F32)
            nc.sync.dma_start(b_sk_sb[:], b_sk[:])
            corr_pad = []
            for c in range(3):
                cp = cpool.tile([KC, H + 2, W + 2], BF16, name=f"corr_pad{c}")
                nc.vector.memset(cp[:, 0, :], 0.0)
                nc.vector.memset(cp[:, H + 1, :], 0.0)
                nc.vector.memset(cp[:, 1:H + 1, 0:1], 0.0)
                nc.vector.memset(cp[:, 1:H + 1, W + 1:W + 2], 0.0)
                stg = spool.tile([KC, HW], BF16, name="stg")
                nc.sync.dma_start(stg[:], corr[c * KC:(c + 1) * KC, :])
                nc.vector.tensor_copy(
                    cp[:, 1:H + 1, 1:W + 1],
                    stg.rearrange("p (h w) -> p h w", h=H))
                corr_pad.append(cp)
            k_sb = cpool.tile([D, HW], BF16)
            nc.sync.dma_start(k_sb[:], k_in[:])
            # vT_sb[p, t, d] = v[d, t*128+p]
            vT_sb = cpool.tile([128, NJ, D], BF16)
            nc.sync.dma_start(vT_sb[:], vT[:])
            wprojT_sb = cpool.tile([D, 2, D], BF16)
            nc.gpsimd.dma_start(wprojT_sb[:], wprojT.rearrange("c p d -> p c d"))
            wf1T_sb = cpool.tile([D, D], BF16)
            nc.gpsimd.dma_start(wf1T_sb[:], wf1T[:])
            wf2T_sb = cpool.tile([D, D], BF16)
            nc.gpsimd.dma_start(wf2T_sb[:], wf2T[:])
            b_proj_sb = cpool.tile([1, D], BF16)
            nc.gpsimd.dma_start(b_proj_sb[:], b_proj[:])
            b_f1_sb = cpool.tile([D, 1], F32)
            nc.gpsimd.dma_start(b_f1_sb[:], b_f1[:])
            b_f2_sb = cpool.tile([1, D], BF16)
            nc.gpsimd.dma_start(b_f2_sb[:], b_f2[:])
            ones_bf = cpool.tile([128, 1], BF16)
            nc.gpsimd.memset(ones_bf[:], 1.0)
            ones_row = cpool.tile([1, TI], BF16)
            nc.gpsimd.memset(ones_row[:], 1.0)
            ones_bf4 = cpool.tile([4, 128], BF16)
            nc.gpsimd.memset(ones_bf4[:], 1.0)
            warm = cpool.tile([128, 128], BF16)
            nc.gpsimd.memset(warm[:], 0.0)
            # HAM warm-up: keep PE busy during the input DMA wait so the
            # first conv runs at 2.4 GHz (clock-gate releases after ~3.4us)
            ps_w = ps_small.tile([128, 128], F32, name="ps_w", tag="sm")
            for _ in range(90):
                nc.tensor.matmul(ps_w[:], warm[:], warm[:],
                                 start=True, stop=True)

            def conv_mm(p):
                """3x3 conv matmuls for i-tiles (2p, 2p+1); weight-paired."""
                ps_c = ps_conv.tile([D, 2, TI], F32, name="ps_c")
                for c in range(3):
                    for t in range(9):
                        dy, dx = t // 3, t % 3
                        for ii in range(2):
                            y0 = (2 * p + ii) * RT
                            nc.tensor.matmul(
                                ps_c[:, ii, :],
                                wskT_sb[:, c * 9 + t, :],
                                corr_pad[c][:, y0 + dy:y0 + dy + RT,
                                            dx:dx + W],
                                start=(c == 0 and t == 0),
                                stop=(c == 2 and t == 8))
                return ps_c

            def conv_evac(ps_c):
                rq = []
                for ii in range(2):
                    resid = qpool.tile([D, TI], F32, name="resid")
                    nc.vector.tensor_scalar_add(resid[:], ps_c[:, ii, :],
                                                b_sk_sb[:])
                    q = qpool.tile([D, TI], BF16, name="q")
                    nc.vector.tensor_copy(q[:], resid[:])
                    rq.append((resid, q))
                return rq

            def s_pair(j, qs):
                ps_sj = []
                for ii in range(2):
                    t = ps_s.tile([128, TI], F32, name="ps_sj", tag="s")
                    nc.tensor.matmul(t[:], k_sb[:, j * 128:(j + 1) * 128],
                                     qs[ii][1][:], start=True, stop=True)
                    ps_sj.append(t)
                return ps_sj

            def normalize_a(ps_m, ii):
                """evacuate + gather the 4 packed denominator rows."""
                m4 = wpool.tile([128, TI], BF16, name="m4")
                nc.vector.tensor_copy(m4[:], ps_m[:])
                m4g = wpool.tile([4, TI], BF16, name="m4g")
                for g in range(4):
                    nc.sync.dma_start(m4g[g:g + 1, :],
                                      m4[32 * g:32 * g + 1, :])
                return m4g

            def normalize_b(m4g, ps_a):
                """merge+broadcast denominators via gathered [2,TI] matmul."""
                ps_b = ps_small.tile([128, TI], F32, name="ps_b", tag="sm")
                nc.tensor.matmul(ps_b[:], ones_bf4[:], m4g[:],
                                 start=True, stop=True)
                rb = wpool.tile([128, TI], F32, name="rb")
                nc.vector.reciprocal(rb[:], ps_b[:])
                attn = wpool.tile([D, TI], BF16, name="attn")
                nc.vector.tensor_mul(attn[:], ps_a[:], rb[:])
                return attn

            def proj(attn, resid, q):
                """1x1 proj on concat([attn, resid]) + bias + resid."""
                ps_p = ps_av.tile([D, TI], F32, name="ps_p", tag="av")
                nc.tensor.matmul(ps_p[:], wprojT_sb[:, 0, :], attn[:],
                                 start=True, stop=False)
                nc.tensor.matmul(ps_p[:], wprojT_sb[:, 1, :], q[:],
                                 start=False, stop=not with_bias)
                if with_bias:
                    nc.tensor.matmul(ps_p[:], b_proj_sb[:], ones_row[:],
                                     start=False, stop=True)
                x = xpool.tile([D, TI], F32, name="x")
                nc.vector.tensor_add(x[:], ps_p[:], resid[:])
                x_bf = xpool.tile([D, TI], BF16, name="x_bf")
                nc.vector.tensor_copy(x_bf[:], x[:])
                return x, x_bf

            def ffn(xv, i):
                x, x_bf = xv
                ps_f1 = ps_s.tile([D, TI], F32, name="ps_f1", tag="s")
                nc.tensor.matmul(ps_f1[:], wf1T_sb[:], x_bf[:],
                                 start=True, stop=True)
                h1 = wpool.tile([D, TI], BF16, name="h1")
                nc.scalar.activation(h1[:], ps_f1[:], AF.Gelu, bias=b_f1_sb[:])
                ps_f2 = ps_s.tile([D, TI], F32, name="ps_f2", tag="s")
                nc.tensor.matmul(ps_f2[:], wf2T_sb[:], h1[:],
                                 start=True, stop=not with_bias)
                if with_bias:
                    nc.tensor.matmul(ps_f2[:], b_f2_sb[:], ones_row[:],
                                     start=False, stop=True)
                o = wpool.tile([D, TI], F32, name="o")
                nc.vector.tensor_add(o[:], ps_f2[:], x[:])
                nc.gpsimd.dma_start(out[:, i * TI:(i + 1) * TI], o[:])

            xs = [None] * NT
            rq_pair = conv_evac(conv_mm(0))
            for p in range(NP):
                i0, i1 = 2 * p, 2 * p + 1
                # ---- attention j-loop, software-pipelined by one j ----
                ps_a0 = ps_av.tile([D, TI], F32, name="ps_a0", tag="av")
                ps_a1 = ps_av.tile([D, TI], F32, name="ps_a1", tag="av")
                ps_sj = s_pair(0, rq_pair)
                e_tiles = [[], []]
                for j in range(NJ):
                    ps_nxt = s_pair(j + 1, rq_pair) if j + 1 < NJ else None
                    for ii, ps_aa in ((0, ps_a0), (1, ps_a1)):
                        e = epool.tile([128, TI], BF16, name="e")
                        nc.scalar.activation(e[:], ps_sj[ii][:], AF.Exp,
                                             scale=SCALE)
                        nc.tensor.matmul(ps_aa[:], vT_sb[:, j, :], e[:],
                                         start=(j == 0), stop=(j == NJ - 1))
                        e_tiles[ii].append(e)
                    ps_sj = ps_nxt

                # ---- softmax denominators: col-packed ones-matmuls ----
                m4gs = []
                for ii in range(2):
                    ps_m = ps_small.tile([128, TI], F32, name="ps_m", tag="sm")
                    for g4 in range(NJ // 4):
                        for g in range(4):
                            nc.tensor.matmul(
                                ps_m[32 * g:32 * g + 1, :], ones_bf[:, 0:1],
                                e_tiles[ii][g4 * 4 + g][:],
                                start=(g4 == 0), stop=(g4 == NJ // 4 - 1),
                                tile_position=(0, 32 * g))
                    m4gs.append(normalize_a(ps_m, ii))

                rq_prev = rq_pair
                if p + 1 < NP:
                    rq_pair = conv_evac(conv_mm(p + 1))

                last = p == NP - 1
                attn0 = normalize_b(m4gs[0], ps_a0)
                if last:
                    ffn(xs[0], 0)
                    ffn(xs[1], 1)
                xs[2 * p] = proj(attn0, rq_prev[0][0], rq_prev[0][1])
                attn1 = normalize_b(m4gs[1], ps_a1)
                if last:
                    ffn(xs[2], 2)
                    ffn(xs[3], 3)
                xs[2 * p + 1] = proj(attn1, rq_prev[1][0], rq_prev[1][1])

            # ---- FFN for the last pair ----
            for i in range(2 * (NP - 1), NT):
                ffn(xs[i], i)

    _split_multi_waits(nc)
    return nc


_NC = {}


def _get_nc(with_bias=True):
    if with_bias not in _NC:
        _NC[with_bias] = build_nc(with_bias)
    return _NC[with_bias]


def _prep_core(corr, k, v, w_sk, b_sk, w_proj, b_proj, w_ffn1, b_ffn1,
               w_ffn2, b_ffn2):
    bf = ml_dtypes.bfloat16
    wskT = np.empty((KC, 27, D), dtype=bf)
    for c in range(3):
        for t in range(9):
            dy, dx = t // 3, t % 3
            wskT[:, c * 9 + t, :] = \
                w_sk[:, c * KC:(c + 1) * KC, dy, dx].T.astype(bf)
    vT = v.reshape(D, HW).T.reshape(NJ, 128, D).transpose(1, 0, 2)
    return {
        "corr": corr.reshape(CIN, HW).astype(bf),
        "k": k.reshape(D, HW).astype(bf),
        "vT": np.ascontiguousarray(vT).astype(bf),
        "wskT": wskT,
        "b_sk": b_sk.reshape(D, 1).astype(np.float32),
        "wprojT": np.ascontiguousarray(
            w_proj.reshape(D, 2 * D).T.reshape(2, D, D)).astype(bf),
        "b_proj": b_proj.reshape(1, D).astype(bf),
        "wf1T": np.ascontiguousarray(w_ffn1.reshape(D, D).T).astype(bf),
        "b_f1": b_ffn1.reshape(D, 1).astype(np.float32),
        "wf2T": np.ascontiguousarray(w_ffn2.reshape(D, D).T).astype(bf),
        "b_f2": b_ffn2.reshape(1, D).astype(bf),
    }


def make_in_maps(corr, k, v, w_sk, b_sk, w_proj, b_proj, w_ffn1, b_ffn1,
                 w_ffn2, b_ffn2):
    corr = np.asarray(corr, dtype=np.float32)
    k = np.asarray(k, dtype=np.float32)
    v = np.asarray(v, dtype=np.float32)
    return [
        _prep_core(corr[i], k[i], v[i], np.asarray(w_sk, np.float32),
                   np.asarray(b_sk, np.float32),
                   np.asarray(w_proj, np.float32),
                   np.asarray(b_proj, np.float32),
                   np.asarray(w_ffn1, np.float32),
                   np.asarray(b_ffn1, np.float32),
                   np.asarray(w_ffn2, np.float32),
                   np.asarray(b_ffn2, np.float32))
        for i in range(N)
    ]


def kernel(corr, k, v, w_sk, b_sk, w_proj, b_proj, w_ffn1, b_ffn1,
           w_ffn2, b_ffn2):
    with_bias = bool(np.any(np.asarray(b_proj)) or np.any(np.asarray(b_ffn2)))
    nc = _get_nc(with_bias)
    in_maps = make_in_maps(corr, k, v, w_sk, b_sk, w_proj, b_proj,
                           w_ffn1, b_ffn1, w_ffn2, b_ffn2)
    res = run_bass_kernel_spmd(nc, in_maps, list(range(N)))
    out = np.stack([res.results[i]["out"].reshape(D, H, W) for i in range(N)])
    return out.astype(np.float32)


# revision 33
# speedup vs baseline: 1.2190x; 1.0632x over previous
"""CostGlobalEncoder TRN2 kernel: conv3x3(324->128) + global HW x HW attention
+ proj + FFN, data-parallel over batch N=8 across 8 NeuronCores.

Self-contained: hardcodes shapes N=8, D=128, H=48, W=64 (HW=3072).
"""
import sys
sys.path.insert(0, '/opt/trn_rl_repo')

import numpy as np
import ml_dtypes

import concourse.bass as bass
import concourse.tile as tile
from concourse import mybir
from concourse.bass_utils import run_bass_kernel_spmd

N, D, H, W = 8, 128, 48, 64
HW = H * W                    # 3072
CIN = 324                     # corr channels
KC = 108                      # conv contraction chunk (324 = 3*108)
NT = 6                        # i-tiles of 512 positions
NP = NT // 2                  # i-tile pairs
TI = 512                      # positions per i-tile
RT = TI // W                  # 8 rows per i-tile
NJ = HW // 128                # 24 j-tiles
SCALE = float(D) ** -0.5

F32 = mybir.dt.float32
BF16 = mybir.dt.bfloat16
AF = mybir.ActivationFunctionType


def _split_multi_waits(nc, max_waits=1):
    """walrus setupSyncWait rejects instructions with several sem-waits;
    hoist extras onto preceding same-engine NOPs (engines run in order)."""
    for fn in nc.m.functions:
        for blk in fn.blocks:
            insts = blk.instructions
            i = 0
            while i < len(insts):
                inst = insts[i]
                si = inst.sync_info
                if si is not None and si.on_wait and len(si.on_wait) > max_waits:
                    waits = list(si.on_wait)
                    extra, keep = waits[:-max_waits], waits[-max_waits:]
                    nops = []
                    while extra:
                        chunk, extra = extra[:max_waits], extra[max_waits:]
                        nop = mybir.InstNoOp(
                            name=f"waitsplit-{nc.next_id()}", ins=[], outs=[])
                        nop.engine = inst.engine
                        nop.sync_info = mybir.SyncInfo(on_wait=chunk, on_update=[])
                        nops.append(nop)
                    inst.sync_info = mybir.SyncInfo(
                        on_wait=keep, on_update=list(si.on_update))
                    blk.instructions = insts = insts[:i] + nops + insts[i:]
                    i += len(nops)
                i += 1


def build_nc(with_bias=True):
    nc = bass.Bass()
    corr = nc.declare_dram_parameter("corr", [CIN, HW], BF16, isOutput=False)
    k_in = nc.declare_dram_parameter("k", [D, HW], BF16, isOutput=False)
    vT = nc.declare_dram_parameter("vT", [128, NJ, D], BF16, isOutput=False)
    wskT = nc.declare_dram_parameter("wskT", [KC, 27, D], BF16, isOutput=False)
    b_sk = nc.declare_dram_parameter("b_sk", [1, D], BF16, isOutput=False)
    wprojT = nc.declare_dram_parameter("wprojT", [2, D, D], BF16, isOutput=False)
    b_proj = nc.declare_dram_parameter("b_proj", [1, D], BF16, isOutput=False)
    wf1T = nc.declare_dram_parameter("wf1T", [D, D], BF16, isOutput=False)
    b_f1 = nc.declare_dram_parameter("b_f1", [D, 1], F32, isOutput=False)
    wf2T = nc.declare_dram_parameter("wf2T", [D, D], BF16, isOutput=False)
    b_f2 = nc.declare_dram_parameter("b_f2", [1, D], BF16, isOutput=False)
    out = nc.declare_dram_parameter("out", [D, HW], F32, isOutput=True)

    with tile.TileContext(nc) as tc:
        with (
            tc.tile_pool(name="const", bufs=1) as cpool,
            tc.tile_pool(name="stage", bufs=2) as spool,
            tc.tile_pool(name="work", bufs=2) as wpool,
            tc.tile_pool(name="qpool", bufs=6) as qpool,
            tc.tile_pool(name="xpool", bufs=7) as xpool,
            tc.tile_pool(name="epool", bufs=27) as epool,
            tc.tile_pool(name="ps_s", bufs=2, space="PSUM") as ps_s,
            tc.tile_pool(name="ps_av", bufs=2, space="PSUM") as ps_av,
            tc.tile_pool(name="ps_conv", bufs=1, space="PSUM") as ps_conv,
        ):
            # ---- load inputs: wskT + corr chunk 0 gate the first matmul ----
            wskT_sb = cpool.tile([KC, 27, D], BF16)
            for c in range(3):
                nc.sync.dma_start(wskT_sb[:, c * 9:(c + 1) * 9, :],
                                  wskT[:, c * 9:(c + 1) * 9, :])
            b_sk_sb = cpool.tile([1, D], BF16)
            nc.sync.dma_start(b_sk_sb[:], b_sk[:])
            corr_pad = []
            for c in range(3):
                cp = cpool.tile([KC, H + 2, W + 2], BF16, name=f"corr_pad{c}")
                nc.vector.memset(cp[:, 0, :], 0.0)
                nc.vector.memset(cp[:, H + 1, :], 0.0)
                nc.vector.memset(cp[:, 1:H + 1, 0:1], 0.0)
                nc.vector.memset(cp[:, 1:H + 1, W + 1:W + 2], 0.0)
                stg = spool.tile([KC, HW], BF16, name="stg")
                nc.sync.dma_start(stg[:], corr[c * KC:(c + 1) * KC, :])
                nc.vector.tensor_copy(
                    cp[:, 1:H + 1, 1:W + 1],
                    stg.rearrange("p (h w) -> p h w", h=H))
                corr_pad.append(cp)
            k_sb = cpool.tile([D, HW], BF16)
            nc.sync.dma_start(k_sb[:], k_in[:])
            # vT_sb[p, t, d] = v[d, t*128+p]
            vT_sb = cpool.tile([128, NJ, D], BF16)
            nc.sync.dma_start(vT_sb[:], vT[:])
            wprojT_sb = cpool.tile([D, 2, D], BF16)
            nc.gpsimd.dma_start(wprojT_sb[:], wprojT.rearrange("c p d -> p c d"))
            wf1T_sb = cpool.tile([D, D], BF16)
            nc.gpsimd.dma_start(wf1T_sb[:], wf1T[:])
            wf2T_sb = cpool.tile([D, D], BF16)
            nc.gpsimd.dma_start(wf2T_sb[:], wf2T[:])
            b_proj_sb = cpool.tile([1, D], BF16)
            nc.gpsimd.dma_start(b_proj_sb[:], b_proj[:])
            b_f1_sb = cpool.tile([D, 1], F32)
            nc.gpsimd.dma_start(b_f1_sb[:], b_f1[:])
            b_f2_sb = cpool.tile([1, D], BF16)
            nc.gpsimd.dma_start(b_f2_sb[:], b_f2[:])
            ones_bf = cpool.tile([128, 1], BF16)
            nc.gpsimd.memset(ones_bf[:], 1.0)
            ones_row = cpool.tile([1, TI], BF16)
            nc.gpsimd.memset(ones_row[:], 1.0)
            ones_bf4 = cpool.tile([4, 128], BF16)
            nc.gpsimd.memset(ones_bf4[:], 1.0)
            warm = cpool.tile([128, 128], BF16)
            nc.gpsimd.memset(warm[:], 0.0)
            # HAM warm-up: keep PE busy during the input DMA wait so the
            # first conv runs at 2.4 GHz (clock-gate releases after ~3.4us)
            ps_w = ps_conv.tile([128, 128], F32, name="ps_w", tag="ps_c")
            for _ in range(90):
                nc.tensor.matmul(ps_w[:], warm[:], warm[:],
                                 start=True, stop=True)

            def conv_mm(p):
                """3x3 conv matmuls for i-tiles (2p, 2p+1); weight-paired."""
                ps_c = ps_conv.tile([D, 2, TI], F32, name="ps_c")
                for c in range(3):
                    for t in range(9):
                        dy, dx = t // 3, t % 3
                        for ii in range(2):
                            y0 = (2 * p + ii) * RT
                            nc.tensor.matmul(
                                ps_c[:, ii, :],
                                wskT_sb[:, c * 9 + t, :],
                                corr_pad[c][:, y0 + dy:y0 + dy + RT,
                                            dx:dx + W],
                                start=(c == 0 and t == 0),
                                stop=(c == 2 and t == 8 and not with_bias))
                if with_bias:
                    for ii in range(2):
                        nc.tensor.matmul(ps_c[:, ii, :], b_sk_sb[:],
                                         ones_row[:], start=False, stop=True)
                return ps_c

            def evac_q(ps_c):
                qs = []
                for ii in range(2):
                    q = qpool.tile([D, TI], BF16, name="q")
                    nc.vector.tensor_copy(q[:], ps_c[:, ii, :])
                    qs.append(q)
                return qs

            def evac_resid(ps_c):
                rs = []
                for ii in range(2):
                    resid = qpool.tile([D, TI], F32, name="resid")
                    nc.vector.tensor_copy(resid[:], ps_c[:, ii, :])
                    rs.append(resid)
                return rs

            def conv_evac(ps_c):
                qs = evac_q(ps_c)
                rs = evac_resid(ps_c)
                return list(zip(rs, qs))

            def s_pair(j, qs):
                t = ps_s.tile([128, 2, TI], F32, name="ps_sj", tag="s")
                for ii in range(2):
                    nc.tensor.matmul(t[:, ii, :],
                                     k_sb[:, j * 128:(j + 1) * 128],
                                     qs[ii][1][:], start=True, stop=True)
                return t

            def normalize_a(ps_m, ii):
                """evacuate + gather the 4 packed denominator rows."""
                m4 = wpool.tile([128, TI], BF16, name="m4")
                nc.vector.tensor_copy(m4[:], ps_m[:])
                m4g = wpool.tile([4, TI], BF16, name="m4g")
                for g in range(4):
                    nc.sync.dma_start(m4g[g:g + 1, :],
                                      m4[32 * g:32 * g + 1, :])
                return m4g

            def normalize_b(m4g, ps_a):
                """merge+broadcast denominators via gathered [2,TI] matmul."""
                ps_b = ps_s.tile([128, TI], F32, name="ps_b", tag="s")
                nc.tensor.matmul(ps_b[:], ones_bf4[:], m4g[:],
                                 start=True, stop=True)
                rb = wpool.tile([128, TI], F32, name="rb")
                nc.vector.reciprocal(rb[:], ps_b[:])
                attn = wpool.tile([D, TI], BF16, name="attn")
                nc.vector.tensor_mul(attn[:], ps_a[:], rb[:])
                return attn

            def proj(attn, resid, q):
                """1x1 proj on concat([attn, resid]) + bias + resid."""
                ps_p = ps_av.tile([D, TI], F32, name="ps_p", tag="av")
                nc.tensor.matmul(ps_p[:], wprojT_sb[:, 0, :], attn[:],
                                 start=True, stop=False)
                nc.tensor.matmul(ps_p[:], wprojT_sb[:, 1, :], q[:],
                                 start=False, stop=not with_bias)
                if with_bias:
                    nc.tensor.matmul(ps_p[:], b_proj_sb[:], ones_row[:],
                                     start=False, stop=True)
                x = xpool.tile([D, TI], F32, name="x")
                nc.vector.tensor_add(x[:], ps_p[:], resid[:])
                x_bf = xpool.tile([D, TI], BF16, name="x_bf")
                nc.vector.tensor_copy(x_bf[:], x[:])
                return x, x_bf

            def ffn(xv, i):
                x, x_bf = xv
                ps_f1 = ps_s.tile([D, TI], F32, name="ps_f1", tag="s")
                nc.tensor.matmul(ps_f1[:], wf1T_sb[:], x_bf[:],
                                 start=True, stop=True)
                h1 = wpool.tile([D, TI], BF16, name="h1")
                nc.scalar.activation(h1[:], ps_f1[:], AF.Gelu, bias=b_f1_sb[:])
                ps_f2 = ps_s.tile([D, TI], F32, name="ps_f2", tag="s")
                nc.tensor.matmul(ps_f2[:], wf2T_sb[:], h1[:],
                                 start=True, stop=not with_bias)
                if with_bias:
                    nc.tensor.matmul(ps_f2[:], b_f2_sb[:], ones_row[:],
                                     start=False, stop=True)
                o = wpool.tile([D, TI], F32, name="o")
                nc.vector.tensor_add(o[:], ps_f2[:], x[:])
                nc.sync.dma_start(out[:, i * TI:(i + 1) * TI], o[:])

            xs = [None] * NT
            rq_pair = conv_evac(conv_mm(0))
            for p in range(NP):
                i0, i1 = 2 * p, 2 * p + 1
                # ---- attention j-loop, software-pipelined by one j ----
                ps_a0 = ps_av.tile([D, TI], F32, name="ps_a0", tag="av")
                ps_a1 = ps_av.tile([D, TI], F32, name="ps_a1", tag="av")
                ps_sj = s_pair(0, rq_pair)
                e_tiles = []
                for j in range(NJ):
                    ps_nxt = s_pair(j + 1, rq_pair) if j + 1 < NJ else None
                    e = epool.tile([128, 2, TI], BF16, name="e")
                    nc.scalar.activation(e[:], ps_sj[:], AF.Exp, scale=SCALE)
                    for ii, ps_aa in ((0, ps_a0), (1, ps_a1)):
                        nc.tensor.matmul(ps_aa[:], vT_sb[:, j, :],
                                         e[:, ii, :],
                                         start=(j == 0), stop=(j == NJ - 1))
                    e_tiles.append(e)
                    ps_sj = ps_nxt

                # ---- softmax denominators: col-packed ones-matmuls ----
                m4gs = []
                for ii in range(2):
                    ps_m = ps_s.tile([128, TI], F32, name="ps_m", tag="s")
                    for g4 in range(NJ // 4):
                        for g in range(4):
                            nc.tensor.matmul(
                                ps_m[32 * g:32 * g + 1, :], ones_bf[:, 0:1],
                                e_tiles[g4 * 4 + g][:, ii, :],
                                start=(g4 == 0), stop=(g4 == NJ // 4 - 1),
                                tile_position=(0, 32 * g))
                    m4gs.append(normalize_a(ps_m, ii))

                rq_prev = rq_pair
                ps_c_next = conv_mm(p + 1) if p + 1 < NP else None
                qs_next = evac_q(ps_c_next) if ps_c_next is not None else None

                last = p == NP - 1
                attn0 = normalize_b(m4gs[0], ps_a0)
                if last:
                    ffn(xs[0], 0)
                    ffn(xs[1], 1)
                xs[2 * p] = proj(attn0, rq_prev[0][0], rq_prev[0][1])
                if ps_c_next is not None:
                    rq_pair = list(zip(evac_resid(ps_c_next), qs_next))
                attn1 = normalize_b(m4gs[1], ps_a1)
                if last:
                    ffn(xs[2], 2)
                    ffn(xs[3], 3)
                xs[2 * p + 1] = proj(attn1, rq_prev[1][0], rq_prev[1][1])

            # ---- FFN for the last pair ----
            for i in range(2 * (NP - 1), NT):
                ffn(xs[i], i)

    _split_multi_waits(nc)
    return nc


_NC = {}


def _get_nc(with_bias=True):
    if with_bias not in _NC:
        _NC[with_bias] = build_nc(with_bias)
    return _NC[with_bias]


def _prep_core(corr, k, v, w_sk, b_sk, w_proj, b_proj, w_ffn1, b_ffn1,
               w_ffn2, b_ffn2):
    bf = ml_dtypes.bfloat16
    wskT = np.empty((KC, 27, D), dtype=bf)
    for c in range(3):
        for t in range(9):
            dy, dx = t // 3, t % 3
            wskT[:, c * 9 + t, :] = \
                w_sk[:, c * KC:(c + 1) * KC, dy, dx].T.astype(bf)
    vT = v.reshape(D, HW).T.reshape(NJ, 128, D).transpose(1, 0, 2)
    return {
        "corr": corr.reshape(CIN, HW).astype(bf),
        "k": k.reshape(D, HW).astype(bf),
        "vT": np.ascontiguousarray(vT).astype(bf),
        "wskT": wskT,
        "b_sk": b_sk.reshape(1, D).astype(bf),
        "wprojT": np.ascontiguousarray(
            w_proj.reshape(D, 2 * D).T.reshape(2, D, D)).astype(bf),
        "b_proj": b_proj.reshape(1, D).astype(bf),
        "wf1T": np.ascontiguousarray(w_ffn1.reshape(D, D).T).astype(bf),
        "b_f1": b_ffn1.reshape(D, 1).astype(np.float32),
        "wf2T": np.ascontiguousarray(w_ffn2.reshape(D, D).T).astype(bf),
        "b_f2": b_ffn2.reshape(1, D).astype(bf),
    }


def make_in_maps(corr, k, v, w_sk, b_sk, w_proj, b_proj, w_ffn1, b_ffn1,
                 w_ffn2, b_ffn2):
    corr = np.asarray(corr, dtype=np.float32)
    k = np.asarray(k, dtype=np.float32)
    v = np.asarray(v, dtype=np.float32)
    return [
        _prep_core(corr[i], k[i], v[i], np.asarray(w_sk, np.float32),
                   np.asarray(b_sk, np.float32),
                   np.asarray(w_proj, np.float32),
                   np.asarray(b_proj, np.float32),
                   np.asarray(w_ffn1, np.float32),
                   np.asarray(b_ffn1, np.float32),
                   np.asarray(w_ffn2, np.float32),
                   np.asarray(b_ffn2, np.float32))
        for i in range(N)
    ]


def kernel(corr, k, v, w_sk, b_sk, w_proj, b_proj, w_ffn1, b_ffn1,
           w_ffn2, b_ffn2):
    with_bias = bool(np.any(np.asarray(b_proj)) or np.any(np.asarray(b_ffn2))
                     or np.any(np.asarray(b_sk)))
    nc = _get_nc(with_bias)
    in_maps = make_in_maps(corr, k, v, w_sk, b_sk, w_proj, b_proj,
                           w_ffn1, b_ffn1, w_ffn2, b_ffn2)
    res = run_bass_kernel_spmd(nc, in_maps, list(range(N)))
    out = np.stack([res.results[i]["out"].reshape(D, H, W) for i in range(N)])
    return out.astype(np.float32)
